# revision 1
# baseline (speedup 1.0000x reference)
"""BondMatrixMessage kernel for 8 TRN2 NeuronCores.

messages[b,e,i] = sum_{k,j} bond_state[b,e,k] * W[k,i,j] * atom_state[b,src_e,j]

Strategy (data-parallel over batch, 4 batches/core):
  - src gather as one-hot matmul on PE (srcT2 = atom2^T @ onehotT)
  - bond broadcast via selector matmuls on PE (rep_c = sel_c @ bondT2)
  - op_c = srcT2 * rep_c elementwise (DVE)  [the per-edge outer product
    bond[e,k]*src[e,j] laid out [(k,j) partitions, e free]]
  - messagesT[i,e] += W2_c^T @ op_c accumulated over 32 (k,j)-chunks (PE)
All host-side work is layout-only (transpose / tile / one-hot scatter).
"""

import sys

sys.path.insert(0, "/opt/trn_rl_repo")

import numpy as np

import concourse.bacc as bacc
import concourse.tile as tile
from concourse import mybir
from concourse.bass_utils import run_bass_kernel_spmd

B, A, E, D, K = 32, 256, 512, 64, 64
NCORES = 8
BPC = B // NCORES          # batches per core
NCHUNK = (K * D) // 128    # 32 contraction chunks of 128
NG = NCHUNK // 2           # 16 chunk pairs

F32 = mybir.dt.float32
F32R = mybir.dt.float32r
BF16 = mybir.dt.bfloat16


def _build(repeat: int = 1, use_f32r: bool = True):
    nc = bacc.Bacc("TRN2", debug=False)
    DT = F32R if use_f32r else F32

    atom2_d = nc.dram_tensor("atom2", [BPC, A, 2 * D], DT, kind="ExternalInput").ap()
    bondT2_d = nc.dram_tensor("bondT2", [BPC, 2 * K, E], DT, kind="ExternalInput").ap()
    onehot_d = nc.dram_tensor("onehotT", [BPC, A, E], DT, kind="ExternalInput").ap()
    w2_d = nc.dram_tensor("w2", [NCHUNK, 128, D], DT, kind="ExternalInput").ap()
    sel_d = nc.dram_tensor("sel", [NG, 128, 128], DT, kind="ExternalInput").ap()
    out_d = nc.dram_tensor("msgT", [BPC, D, E], F32, kind="ExternalOutput").ap()

    with tile.TileContext(nc) as tc:
        with (
            tc.tile_pool(name="consts", bufs=1) as consts,
            tc.tile_pool(name="inp", bufs=2) as inp,
            tc.tile_pool(name="work", bufs=6) as work,
            tc.tile_pool(name="ps_src", bufs=1, space="PSUM") as ps_src,
            tc.tile_pool(name="ps_rep", bufs=6, space="PSUM") as ps_rep,
            tc.tile_pool(name="ps_out", bufs=1, space="PSUM") as ps_out,
        ):
            # constants: W2 chunks + selector tiles (loaded once)
            w2_t = []
            for c in range(NCHUNK):
                t = consts.tile([128, D], DT, tag=f"w2_{c}")
                nc.sync.dma_start(t[:], w2_d[c])
                w2_t.append(t)
            sel_t = []
            for g in range(NG):
                t = consts.tile([128, 128], DT, tag=f"sel_{g}")
                nc.sync.dma_start(t[:], sel_d[g])
                sel_t.append(t)

            def start_batch(b):
                """DMA loads + src gather matmuls for one batch; returns
                per-batch state (srcT2, bt2, mout)."""
                at0 = inp.tile([128, 2 * D], DT, tag="at0")
                nc.sync.dma_start(at0[:], atom2_d[b, 0:128, :])
                at1 = inp.tile([128, 2 * D], DT, tag="at1")
                nc.sync.dma_start(at1[:], atom2_d[b, 128:256, :])
                bt2 = inp.tile([2 * K, E], DT, tag="bt2")
                nc.sync.dma_start(bt2[:], bondT2_d[b])
                oh0 = inp.tile([128, E], DT, tag="oh0")
                nc.sync.dma_start(oh0[:], onehot_d[b, 0:128, :])
                oh1 = inp.tile([128, E], DT, tag="oh1")
                nc.sync.dma_start(oh1[:], onehot_d[b, 128:256, :])

                # srcT2[p=(h,j), e] = src[e, j] (duplicated over h)
                ps = ps_src.tile([128, E], F32, tag="ps_src")
                nc.tensor.matmul(ps[:], at0[:], oh0[:],
                                 start=True, stop=False)
                nc.tensor.matmul(ps[:], at1[:], oh1[:],
                                 start=False, stop=True)
                srcT2 = work.tile([128, E], DT, tag="srcT2")
                nc.scalar.activation(srcT2[:], ps[:],
                                     mybir.ActivationFunctionType.Copy)
                mout = ps_out.tile([D, E], F32, tag="mout")
                return srcT2, bt2, mout, b

            def do_rep(st, g):
                _, bt2, _, _ = st
                rep_e = ps_rep.tile([128, E], F32, tag="rep")
                nc.tensor.matmul(rep_e[:], sel_t[g][0:64, :],
                                 bt2[0:64, :],
                                 start=True, stop=True,
                                 tile_position=(0, 0))
                rep_o = ps_rep.tile([128, E], F32, tag="rep")
                nc.tensor.matmul(rep_o[:], sel_t[g][64:128, :],
                                 bt2[64:128, :],
                                 start=True, stop=True,
                                 tile_position=(64, 0))
                return rep_e, rep_o

            def do_pair(st, g, reps):
                srcT2, _, mout, _ = st
                rep_e, rep_o = reps
                for half, rep in ((0, rep_e), (1, rep_o)):
                    c = 2 * g + half
                    op_t = work.tile([128, E], DT, tag="op")
                    if c % 8 in (1, 4, 6):
                        # offload 12/32 Hadamards to GPSIMD (~2x slower
                        # than DVE; rep staged to SBUF via the idle ACT
                        # since Pool can't read PSUM)
                        rep_s = work.tile([128, E], F32, tag="repc")
                        nc.scalar.activation(
                            rep_s[:], rep[:],
                            mybir.ActivationFunctionType.Copy)
                        nc.gpsimd.tensor_mul(op_t[:], srcT2[:], rep_s[:])
                    else:
                        nc.vector.tensor_mul(op_t[:], srcT2[:], rep[:])
                    nc.tensor.matmul(mout[:], w2_t[c][:], op_t[:],
                                     start=(c == 0), stop=(c == NCHUNK - 1))

            def finish_batch(st):
                _, _, mout, b = st
                res = work.tile([D, E], F32, tag="res")
                nc.scalar.activation(res[:], mout[:],
                                     mybir.ActivationFunctionType.Copy)
                nc.sync.dma_start(out_d[b], res[:])

            def body(_=None):
                # staggered batches: next batch's DMAs, src-gather matmuls
                # and first rep pair are issued near the end of the current
                # batch's chunk loop so the PE/DVE never drain between
                # batches
                from collections import deque
                sts = {0: None}
                fifos = {}

                def launch(b):
                    st = start_batch(b)
                    f = deque()
                    f.append((0, do_rep(st, 0)))
                    f.append((1, do_rep(st, 1)))
                    sts[b] = st
                    fifos[b] = f

                launch(0)
                for b in range(BPC):
                    st = sts[b]
                    fifo = fifos[b]
                    for g in range(0, NG, 2):
                        # cluster two rep pairs (re,ro,re,ro) then four mains
                        # so rep weight loads hide behind rep streams
                        for gg2 in (g + 2, g + 3):
                            if gg2 < NG:
                                fifo.append((gg2, do_rep(st, gg2)))
                        if g == NG - 4 and b + 1 < BPC:
                            launch(b + 1)
                        for _ in range(2):
                            gg, reps = fifo.popleft()
                            do_pair(st, gg, reps)
                    finish_batch(st)

            if repeat == 1:
                body()
            else:
                tc.For_i_unrolled_general(
                    0, repeat, 1,
                    lambda iv, unroll: body(iv), 1,
                    hint_engines=(mybir.EngineType.PE, mybir.EngineType.DVE,
                                  mybir.EngineType.Activation,
                                  mybir.EngineType.Pool, mybir.EngineType.SP))

    nc.compile()
    return nc


_CACHE = {}


def _get_nc(repeat=1, use_f32r=True):
    key = (repeat, use_f32r)
    if key not in _CACHE:
        _CACHE[key] = _build(repeat, use_f32r)
    return _CACHE[key]


def _prep_core_inputs(atom_state, bond_state, connectivity):
    """Host-side layout prep for one core's batch slice (no float math)."""
    bpc = atom_state.shape[0]
    atom2 = np.concatenate([atom_state, atom_state], axis=2)          # [b,A,2D]
    atom2 = np.ascontiguousarray(atom2, dtype=np.float32)
    bT = np.swapaxes(bond_state, 1, 2)                                # [b,K,E]
    bondT2 = np.concatenate([bT, bT], axis=1)                         # [b,2K,E]
    bondT2 = np.ascontiguousarray(bondT2, dtype=np.float32)
    idx = connectivity[:, :, 0].astype(np.int64)                      # [b,E]
    onehotT = (idx[:, None, :] == np.arange(A)[None, :, None])
    onehotT = np.ascontiguousarray(onehotT, dtype=np.float32)         # [b,A,E]
    assert atom2.shape == (bpc, A, 2 * D)
    return atom2, bondT2, onehotT


def _shared_inputs(bond_transform):
    w2 = bond_transform.reshape(K, D, D).transpose(0, 2, 1).reshape(K * D, D)
    w2 = np.ascontiguousarray(w2.reshape(NCHUNK, 128, D), dtype=np.float32)
    sel = np.zeros((NG, 128, 128), dtype=np.float32)
    for g in range(NG):
        for h in range(2):
            # rows 0:64  -> chunk 2g   : k = 4g + h
            sel[g, 4 * g + h, h * D:(h + 1) * D] = 1.0
            # rows 64:128-> chunk 2g+1 : k = 4g + 2 + h
            sel[g, 64 + 4 * g + 2 + h, h * D:(h + 1) * D] = 1.0
    return w2, sel


def kernel(atom_state, bond_state, connectivity, bond_transform,
           repeat=1, use_f32r=True):
    atom_state = np.asarray(atom_state, dtype=np.float32)
    bond_state = np.asarray(bond_state, dtype=np.float32)
    connectivity = np.asarray(connectivity)
    bond_transform = np.asarray(bond_transform, dtype=np.float32)

    nc = _get_nc(repeat, use_f32r)
    w2, sel = _shared_inputs(bond_transform)

    in_maps = []
    for m in range(NCORES):
        sl = slice(m * BPC, (m + 1) * BPC)
        atom2, bondT2, onehotT = _prep_core_inputs(
            atom_state[sl], bond_state[sl], connectivity[sl])
        in_maps.append({
            "atom2": atom2,
            "bondT2": bondT2,
            "onehotT": onehotT,
            "w2": w2,
            "sel": sel,
        })

    res = run_bass_kernel_spmd(nc, in_maps, list(range(NCORES)))

    out = np.empty((B, E, D), dtype=np.float32)
    for m in range(NCORES):
        msgT = res.results[m]["msgT"]                                  # [BPC,D,E]
        out[m * BPC:(m + 1) * BPC] = np.swapaxes(msgT, 1, 2)
    return out


if __name__ == "__main__":
    rng = np.random.default_rng(0)
    atom = rng.standard_normal((B, A, D)).astype(np.float32)
    bond = rng.standard_normal((B, E, K)).astype(np.float32)
    conn = rng.integers(0, A, size=(B, E, 2)).astype(np.int64)
    bt = rng.standard_normal((K, D * D)).astype(np.float32) * 0.01
    out = kernel(atom, bond, conn, bt)
    print("out", out.shape, out.dtype, float(np.abs(out).max()))



# revision 2
# speedup vs baseline: 1.3728x; 1.3728x over previous
"""BondMatrixMessage kernel for 8 TRN2 NeuronCores — v2.

messages[b,e,i] = sum_{k,j} bond_state[b,e,k] * W[k,i,j] * atom_state[b,src_e,j]

v2 restructures the (k,j) contraction chunks as 8 k's x 16 j's (v1: 2 x 64):
  - bond broadcast: 8 rep matmuls/batch (one [128,512] tile = 8 k's x 16dup
    serves 4 chunks), issued as 4 quadrant pairs into [128,1024] PSUM tiles
  - src gather: 8 one-hot matmuls/batch -> 4 srcR tiles (16 j's x 8dup),
    copied to SBUF bf16 in two [128,1024] ACT copies
  - every rep pair staged once to SBUF bf16 (ACT); Hadamards run 2x bf16
    on DVE ('A') or on GPSIMD ('P'); per-chunk paths tunable
  - mains: 32 accumulating K=128 matmuls into mout[64,512]
  - PE per batch: 8 + 8 + 32 = 48 matmuls (v1: 66)
  - all SBUF bf16; output bf16, upcast on host

Chunk c = a*4 + d covers k in [8a, 8a+8) x j in [16d, 16d+16),
row layout (i, m) -> k = 8a+i, j = 16d+m.
"""

import sys

sys.path.insert(0, "/opt/trn_rl_repo")

import ml_dtypes
import numpy as np

import concourse.bacc as bacc
import concourse.tile as tile
from concourse import mybir
from concourse.bass_utils import run_bass_kernel_spmd

B, A, E, D, K = 32, 256, 512, 64, 64
NCORES = 8
BPC = B // NCORES
NCHUNK = 32     # (k,j) chunks of 128 rows = 8 k x 16 j
NKB = 8         # k-blocks (a = 0..7)
NJB = 4         # j-blocks (d = 0..3)
NPAIR = NKB // 2

F32 = mybir.dt.float32
BF16 = mybir.dt.bfloat16
BF16NP = ml_dtypes.bfloat16

# per-chunk Hadamard engine, chunk c = a*4 + d: 'A' DVE bf16, 'P' Pool, 'D' DVE-from-PSUM
PATHS_DEFAULT = "AAAP" * 8


def _build(repeat=1, paths=PATHS_DEFAULT, deep=True):
    assert len(paths) == NCHUNK
    nc = bacc.Bacc("TRN2", debug=False)

    # atom8[b, p, q*512 + d*128 + i*16 + m] = atom[b, q*128+p, 16d+m]
    atom8_d = nc.dram_tensor("atom8", [BPC, 128, 1024], BF16,
                             kind="ExternalInput").ap()
    bondT2_d = nc.dram_tensor("bondT2", [BPC, 128, E], BF16,
                              kind="ExternalInput").ap()
    onehot_d = nc.dram_tensor("onehotT", [BPC, 128, 2 * E], BF16,
                              kind="ExternalInput").ap()
    w2_d = nc.dram_tensor("w2", [128, NCHUNK * D], BF16,
                          kind="ExternalInput").ap()
    sel_d = nc.dram_tensor("sel", [128, NPAIR * 128], BF16,
                           kind="ExternalInput").ap()
    out_d = nc.dram_tensor("msgT", [BPC, D, E], BF16, kind="ExternalOutput").ap()

    with tile.TileContext(nc) as tc:
        with (
            tc.tile_pool(name="consts", bufs=1) as consts,
            tc.tile_pool(name="inp", bufs=3 if deep else 2) as inp,
            tc.tile_pool(name="work", bufs=8 if deep else 6) as work,
            tc.tile_pool(name="ps2", bufs=3 if deep else 2, space="PSUM") as ps2,
            tc.tile_pool(name="ps_out", bufs=1, space="PSUM") as ps_out,
        ):
            w2_all = consts.tile([128, NCHUNK * D], BF16, tag="w2")
            nc.sync.dma_start(w2_all[:], w2_d)
            sel_all = consts.tile([128, NPAIR * 128], BF16, tag="sel")
            nc.sync.dma_start(sel_all[:], sel_d)
            w2_t = [w2_all[:, c * D:(c + 1) * D] for c in range(NCHUNK)]
            sel_t = [sel_all[:, t * 128:(t + 1) * 128] for t in range(NPAIR)]

            def start_batch(b):
                atP = inp.tile([128, 1024], BF16, tag="atP")
                nc.sync.dma_start(atP[:], atom8_d[b])
                bt2 = inp.tile([128, E], BF16, tag="bt2")
                nc.sync.dma_start(bt2[:], bondT2_d[b])
                ohP = inp.tile([128, 2 * E], BF16, tag="ohP")
                nc.sync.dma_start(ohP[:], onehot_d[b])

                # srcR[:, d*E:(d+1)*E][(i,m), e] = src[e, 16d+m] (dup over i)
                srcR = work.tile([128, NJB * E], BF16, tag="srcR")
                for dd in (0, 2):
                    ps = ps2.tile([128, 2 * E], F32, tag="ps2")
                    for d in (dd, dd + 1):
                        o = (d - dd) * E
                        for q in (0, 1):
                            nc.tensor.matmul(
                                ps[:, o:o + E],
                                atP[:, q * 512 + d * 128:q * 512 + (d + 1) * 128],
                                ohP[:, q * E:(q + 1) * E],
                                start=(q == 0), stop=(q == 1))
                    nc.scalar.activation(srcR[:, dd * E:(dd + 2) * E], ps[:],
                                         mybir.ActivationFunctionType.Copy)
                mout = ps_out.tile([D, E], F32, tag="mout")
                return srcR, bt2, mout, b

            def do_rep(st, t):
                """k-block pair (a=2t, 2t+1): two quadrant matmuls into one
                [128,1024] PSUM tile + one staged bf16 copy."""
                _, bt2, _, _ = st
                rep2 = ps2.tile([128, 2 * E], F32, tag="ps2")
                nc.tensor.matmul(rep2[:, 0:E], sel_t[t][0:64, :], bt2[0:64, :],
                                 start=True, stop=True, tile_position=(0, 0))
                nc.tensor.matmul(rep2[:, E:2 * E], sel_t[t][64:128, :],
                                 bt2[64:128, :],
                                 start=True, stop=True, tile_position=(64, 0))
                rep_s = work.tile([128, 2 * E], BF16, tag="rep_s")
                nc.scalar.activation(rep_s[:], rep2[:],
                                     mybir.ActivationFunctionType.Copy)
                return rep2, rep_s

            def do_block(st, t, reps, half):
                """All 4 chunks of k-block a = 2t+half."""
                srcR, _, mout, _ = st
                rep2, rep_s = reps
                a = 2 * t + half
                for d in range(NJB):
                    c = a * NJB + d
                    path = paths[c]
                    op_t = work.tile([128, E], BF16, tag="op")
                    if path == "A":
                        nc.vector.tensor_mul(op_t[:], srcR[:, d * E:(d + 1) * E],
                                             rep_s[:, half * E:(half + 1) * E])
                    elif path == "P":
                        nc.gpsimd.tensor_mul(op_t[:], srcR[:, d * E:(d + 1) * E],
                                             rep_s[:, half * E:(half + 1) * E])
                    else:
                        nc.vector.tensor_mul(op_t[:], srcR[:, d * E:(d + 1) * E],
                                             rep2[:, half * E:(half + 1) * E])
                    nc.tensor.matmul(mout[:], w2_t[c], op_t[:],
                                     start=(c == 0), stop=(c == NCHUNK - 1))

            def finish_batch(st):
                _, _, mout, b = st
                res = work.tile([D, E], BF16, tag="res")
                nc.scalar.activation(res[:], mout[:],
                                     mybir.ActivationFunctionType.Copy)
                nc.sync.dma_start(out_d[b], res[:])

            def body_n(n):
                """n iterations' worth of batches, software-pipelined as one
                sweep (batch g maps to dram slot g % BPC)."""
                from collections import deque
                nb = BPC * n
                sts = {}
                fifos = {}

                def launch(g):
                    st = start_batch(g % BPC)
                    f = deque()
                    f.append((0, do_rep(st, 0)))
                    if deep:
                        f.append((1, do_rep(st, 1)))
                    sts[g] = st
                    fifos[g] = f

                launch(0)
                for g in range(nb):
                    st = sts[g]
                    fifo = fifos[g]
                    for t in range(NPAIR):
                        ahead = t + 2 if deep else t + 1
                        if ahead < NPAIR and (deep or True):
                            if ahead == t + 1 or ahead < NPAIR:
                                fifo.append((ahead, do_rep(st, ahead)))
                        if t == NPAIR - 2 and g + 1 < nb:
                            launch(g + 1)
                        tt, reps = fifo.popleft()
                        do_block(st, tt, reps, 0)
                        do_block(st, tt, reps, 1)
                    finish_batch(st)
                    del sts[g], fifos[g]

            if repeat == 1:
                body_n(1)
            else:
                unroll = 4 if repeat % 4 == 0 else 1
                tc.For_i_unrolled_general(
                    0, repeat, 1,
                    lambda iv, u: body_n(u), unroll,
                    hint_engines=(mybir.EngineType.PE, mybir.EngineType.DVE,
                                  mybir.EngineType.Activation,
                                  mybir.EngineType.Pool, mybir.EngineType.SP))

    nc.compile()
    return nc


_CACHE = {}


def _get_nc(repeat=1, paths=PATHS_DEFAULT, deep=True):
    key = (repeat, paths, deep)
    if key not in _CACHE:
        _CACHE[key] = _build(repeat, paths, deep)
    return _CACHE[key]


def _prep_core_inputs(atom_state, bond_state, connectivity):
    bpc = atom_state.shape[0]
    # atom8[b, p, q*512 + d*128 + i*16 + m] = atom[b, q*128+p, 16d+m]
    a8 = atom_state.reshape(bpc, 2, 128, NJB, 16)            # [b, q, p, d, m]
    a8 = np.broadcast_to(a8[:, :, :, :, None, :],
                         (bpc, 2, 128, NJB, 8, 16))          # dup over i
    a8 = a8.transpose(0, 2, 1, 3, 4, 5).reshape(bpc, 128, 1024)
    atom8 = np.ascontiguousarray(a8.astype(BF16NP))

    bT = np.swapaxes(bond_state, 1, 2)                       # [b, K, E]
    bondT2 = np.concatenate([bT, bT], axis=1)                # [b, 128, E]
    bondT2 = np.ascontiguousarray(bondT2.astype(BF16NP))

    idx = connectivity[:, :, 0].astype(np.int64)             # [b, E]
    onehotT = (idx[:, None, :] == np.arange(A)[None, :, None])  # [b, A, E]
    onehotT = onehotT.reshape(bpc, 2, 128, E).transpose(0, 2, 1, 3)
    onehotT = np.ascontiguousarray(onehotT.reshape(bpc, 128, 2 * E).astype(BF16NP))
    return atom8, bondT2, onehotT


def _shared_inputs(bond_transform):
    W = bond_transform.reshape(K, D, D)                      # [k, i', j]
    W_perm = W.transpose(0, 2, 1)                            # [k, j, i']
    w2v2 = np.empty((NCHUNK, 128, D), dtype=np.float32)
    for c in range(NCHUNK):
        a, d = c // NJB, c % NJB
        w2v2[c] = W_perm[8 * a:8 * a + 8, 16 * d:16 * d + 16, :].reshape(128, D)
    w2 = w2v2.transpose(1, 0, 2).reshape(128, NCHUNK * D)
    w2 = np.ascontiguousarray(w2.astype(BF16NP))

    sel = np.zeros((NPAIR, 128, 128), dtype=BF16NP)
    for t in range(NPAIR):
        for half in range(2):
            a = 2 * t + half
            for i in range(8):
                for m in range(16):
                    sel[t, half * 64 + 8 * a + i, i * 16 + m] = 1.0
    sel = np.ascontiguousarray(sel.transpose(1, 0, 2).reshape(128, NPAIR * 128))
    return w2, sel


def kernel(atom_state, bond_state, connectivity, bond_transform,
           repeat=1, paths=PATHS_DEFAULT, deep=True):
    atom_state = np.asarray(atom_state, dtype=np.float32)
    bond_state = np.asarray(bond_state, dtype=np.float32)
    connectivity = np.asarray(connectivity)
    bond_transform = np.asarray(bond_transform, dtype=np.float32)

    nc = _get_nc(repeat, paths, deep)
    w2, sel = _shared_inputs(bond_transform)

    in_maps = []
    for m in range(NCORES):
        sl = slice(m * BPC, (m + 1) * BPC)
        atom8, bondT2, onehotT = _prep_core_inputs(
            atom_state[sl], bond_state[sl], connectivity[sl])
        in_maps.append({
            "atom8": atom8,
            "bondT2": bondT2,
            "onehotT": onehotT,
            "w2": w2,
            "sel": sel,
        })

    res = run_bass_kernel_spmd(nc, in_maps, list(range(NCORES)))

    out = np.empty((B, E, D), dtype=np.float32)
    for m in range(NCORES):
        msgT = np.asarray(res.results[m]["msgT"], dtype=np.float32)
        out[m * BPC:(m + 1) * BPC] = np.swapaxes(msgT, 1, 2)
    return out


if __name__ == "__main__":
    rng = np.random.default_rng(0)
    atom = rng.standard_normal((B, A, D)).astype(np.float32)
    bond = rng.standard_normal((B, E, K)).astype(np.float32)
    conn = rng.integers(0, A, size=(B, E, 2)).astype(np.int64)
    bt = rng.standard_normal((K, D * D)).astype(np.float32) * 0.01
    out = kernel(atom, bond, conn, bt)
    print("out", out.shape, out.dtype, float(np.abs(out).max()))


# revision 4
# speedup vs baseline: 1.6642x; 1.2123x over previous
"""BondMatrixMessage kernel for 8 TRN2 NeuronCores — v2.

messages[b,e,i] = sum_{k,j} bond_state[b,e,k] * W[k,i,j] * atom_state[b,src_e,j]

v2 restructures the (k,j) contraction chunks as 8 k's x 16 j's (v1: 2 x 64):
  - bond broadcast: 8 rep matmuls/batch (one [128,512] tile = 8 k's x 16dup
    serves 4 chunks), issued as 4 quadrant pairs into [128,1024] PSUM tiles
  - src gather: 8 one-hot matmuls/batch -> 4 srcR tiles (16 j's x 8dup),
    copied to SBUF bf16 in two [128,1024] ACT copies
  - every rep pair staged once to SBUF bf16 (ACT); all Hadamards run as
    2x bf16 tensor_tensor on DVE (GPSIMD proved ~20us/iter slower on HW:
    shared SBUF port + slow Q7 muls serialize the main-matmul chain)
  - mains: 32 accumulating K=128 matmuls into mout[64,512]
  - PE per batch: 8 + 8 + 32 = 48 matmuls (v1: 66)
  - all SBUF bf16; output bf16, upcast on host

Chunk c = a*4 + d covers k in [8a, 8a+8) x j in [16d, 16d+16),
row layout (i, m) -> k = 8a+i, j = 16d+m.
"""

import sys

sys.path.insert(0, "/opt/trn_rl_repo")

import ml_dtypes
import numpy as np

import concourse.bacc as bacc
import concourse.tile as tile
from concourse import mybir
from concourse.bass_utils import run_bass_kernel_spmd

B, A, E, D, K = 32, 256, 512, 64, 64
NCORES = 8
BPC = B // NCORES
NCHUNK = 32     # (k,j) chunks of 128 rows = 8 k x 16 j
NKB = 8         # k-blocks (a = 0..7)
NJB = 4         # j-blocks (d = 0..3)
NPAIR = NKB // 2

F32 = mybir.dt.float32
BF16 = mybir.dt.bfloat16
BF16NP = ml_dtypes.bfloat16

# per-chunk Hadamard engine, chunk c = a*4 + d: 'A' DVE bf16, 'P' Pool, 'D' DVE-from-PSUM
PATHS_DEFAULT = "A" * 32


def _build(repeat=1, paths=PATHS_DEFAULT, deep=True):
    assert len(paths) == NCHUNK
    nc = bacc.Bacc("TRN2", debug=False)

    # atom8[b, p, q*512 + d*128 + i*16 + m] = atom[b, q*128+p, 16d+m]
    atom8_d = nc.dram_tensor("atom8", [BPC, 128, 1024], BF16,
                             kind="ExternalInput").ap()
    bondT2_d = nc.dram_tensor("bondT2", [BPC, 128, E], BF16,
                              kind="ExternalInput").ap()
    onehot_d = nc.dram_tensor("onehotT", [BPC, 128, 2 * E], BF16,
                              kind="ExternalInput").ap()
    w2_d = nc.dram_tensor("w2", [128, NCHUNK * D], BF16,
                          kind="ExternalInput").ap()
    sel_d = nc.dram_tensor("sel", [128, NPAIR * 128], BF16,
                           kind="ExternalInput").ap()
    out_d = nc.dram_tensor("msgT", [BPC, D, E], BF16, kind="ExternalOutput").ap()

    with tile.TileContext(nc) as tc:
        with (
            tc.tile_pool(name="consts", bufs=1) as consts,
            tc.tile_pool(name="inp", bufs=3 if deep else 2) as inp,
            tc.tile_pool(name="work", bufs=8 if deep else 6) as work,
            tc.tile_pool(name="ps2", bufs=3 if deep else 2, space="PSUM") as ps2,
            tc.tile_pool(name="ps_out", bufs=1, space="PSUM") as ps_out,
        ):
            w2_all = consts.tile([128, NCHUNK * D], BF16, tag="w2")
            nc.sync.dma_start(w2_all[:], w2_d)
            sel_all = consts.tile([128, NPAIR * 128], BF16, tag="sel")
            nc.sync.dma_start(sel_all[:], sel_d)
            w2_t = [w2_all[:, c * D:(c + 1) * D] for c in range(NCHUNK)]
            sel_t = [sel_all[:, t * 128:(t + 1) * 128] for t in range(NPAIR)]

            def start_batch(b):
                atP = inp.tile([128, 1024], BF16, tag="atP")
                nc.sync.dma_start(atP[:], atom8_d[b])
                bt2 = inp.tile([128, E], BF16, tag="bt2")
                nc.sync.dma_start(bt2[:], bondT2_d[b])
                ohP = inp.tile([128, 2 * E], BF16, tag="ohP")
                nc.sync.dma_start(ohP[:], onehot_d[b])

                # srcR[:, d*E:(d+1)*E][(i,m), e] = src[e, 16d+m] (dup over i)
                srcR = work.tile([128, NJB * E], BF16, tag="srcR")
                for dd in (0, 2):
                    ps = ps2.tile([128, 2 * E], F32, tag="ps2")
                    for d in (dd, dd + 1):
                        o = (d - dd) * E
                        for q in (0, 1):
                            nc.tensor.matmul(
                                ps[:, o:o + E],
                                atP[:, q * 512 + d * 128:q * 512 + (d + 1) * 128],
                                ohP[:, q * E:(q + 1) * E],
                                start=(q == 0), stop=(q == 1))
                    nc.scalar.activation(srcR[:, dd * E:(dd + 2) * E], ps[:],
                                         mybir.ActivationFunctionType.Copy)
                mout = ps_out.tile([D, E], F32, tag="mout")
                return srcR, bt2, mout, b

            def do_rep(st, t):
                """k-block pair (a=2t, 2t+1): two quadrant matmuls into one
                [128,1024] PSUM tile + one staged bf16 copy."""
                _, bt2, _, _ = st
                rep2 = ps2.tile([128, 2 * E], F32, tag="ps2")
                nc.tensor.matmul(rep2[:, 0:E], sel_t[t][0:64, :], bt2[0:64, :],
                                 start=True, stop=True, tile_position=(0, 0))
                nc.tensor.matmul(rep2[:, E:2 * E], sel_t[t][64:128, :],
                                 bt2[64:128, :],
                                 start=True, stop=True, tile_position=(64, 0))
                rep_s = work.tile([128, 2 * E], BF16, tag="rep_s")
                nc.scalar.activation(rep_s[:], rep2[:],
                                     mybir.ActivationFunctionType.Copy)
                return rep2, rep_s

            def do_block(st, t, reps, half):
                """All 4 chunks of k-block a = 2t+half."""
                srcR, _, mout, _ = st
                rep2, rep_s = reps
                a = 2 * t + half
                for d in range(NJB):
                    c = a * NJB + d
                    path = paths[c]
                    op_t = work.tile([128, E], BF16, tag="op")
                    if path == "A":
                        nc.vector.tensor_mul(op_t[:], srcR[:, d * E:(d + 1) * E],
                                             rep_s[:, half * E:(half + 1) * E])
                    elif path == "P":
                        nc.gpsimd.tensor_mul(op_t[:], srcR[:, d * E:(d + 1) * E],
                                             rep_s[:, half * E:(half + 1) * E])
                    else:
                        nc.vector.tensor_mul(op_t[:], srcR[:, d * E:(d + 1) * E],
                                             rep2[:, half * E:(half + 1) * E])
                    nc.tensor.matmul(mout[:], w2_t[c], op_t[:],
                                     start=(c == 0), stop=(c == NCHUNK - 1))

            def finish_batch(st):
                _, _, mout, b = st
                res = work.tile([D, E], BF16, tag="res")
                nc.scalar.activation(res[:], mout[:],
                                     mybir.ActivationFunctionType.Copy)
                nc.sync.dma_start(out_d[b], res[:])

            def body_n(n):
                """n iterations' worth of batches, software-pipelined as one
                sweep (batch g maps to dram slot g % BPC)."""
                from collections import deque
                nb = BPC * n
                sts = {}
                fifos = {}

                def launch(g):
                    st = start_batch(g % BPC)
                    f = deque()
                    f.append((0, do_rep(st, 0)))
                    if deep:
                        f.append((1, do_rep(st, 1)))
                    sts[g] = st
                    fifos[g] = f

                launch(0)
                for g in range(nb):
                    st = sts[g]
                    fifo = fifos[g]
                    for t in range(NPAIR):
                        ahead = t + 2 if deep else t + 1
                        if ahead < NPAIR and (deep or True):
                            if ahead == t + 1 or ahead < NPAIR:
                                fifo.append((ahead, do_rep(st, ahead)))
                        if t == NPAIR - 2 and g + 1 < nb:
                            launch(g + 1)
                        tt, reps = fifo.popleft()
                        do_block(st, tt, reps, 0)
                        do_block(st, tt, reps, 1)
                    finish_batch(st)
                    del sts[g], fifos[g]

            if repeat == 1:
                body_n(1)
            else:
                unroll = (8 if repeat % 8 == 0 else
                          4 if repeat % 4 == 0 else 1)
                tc.For_i_unrolled_general(
                    0, repeat, 1,
                    lambda iv, u: body_n(u), unroll,
                    hint_engines=(mybir.EngineType.PE, mybir.EngineType.DVE,
                                  mybir.EngineType.Activation,
                                  mybir.EngineType.Pool, mybir.EngineType.SP))

    nc.compile()
    return nc


_CACHE = {}


def _get_nc(repeat=1, paths=PATHS_DEFAULT, deep=True):
    key = (repeat, paths, deep)
    if key not in _CACHE:
        _CACHE[key] = _build(repeat, paths, deep)
    return _CACHE[key]


def _prep_core_inputs(atom_state, bond_state, connectivity):
    bpc = atom_state.shape[0]
    # atom8[b, p, q*512 + d*128 + i*16 + m] = atom[b, q*128+p, 16d+m]
    a8 = atom_state.reshape(bpc, 2, 128, NJB, 16)            # [b, q, p, d, m]
    a8 = np.broadcast_to(a8[:, :, :, :, None, :],
                         (bpc, 2, 128, NJB, 8, 16))          # dup over i
    a8 = a8.transpose(0, 2, 1, 3, 4, 5).reshape(bpc, 128, 1024)
    atom8 = np.ascontiguousarray(a8.astype(BF16NP))

    bT = np.swapaxes(bond_state, 1, 2)                       # [b, K, E]
    bondT2 = np.concatenate([bT, bT], axis=1)                # [b, 128, E]
    bondT2 = np.ascontiguousarray(bondT2.astype(BF16NP))

    idx = connectivity[:, :, 0].astype(np.int64)             # [b, E]
    onehotT = (idx[:, None, :] == np.arange(A)[None, :, None])  # [b, A, E]
    onehotT = onehotT.reshape(bpc, 2, 128, E).transpose(0, 2, 1, 3)
    onehotT = np.ascontiguousarray(onehotT.reshape(bpc, 128, 2 * E).astype(BF16NP))
    return atom8, bondT2, onehotT


def _shared_inputs(bond_transform):
    W = bond_transform.reshape(K, D, D)                      # [k, i', j]
    W_perm = W.transpose(0, 2, 1)                            # [k, j, i']
    w2v2 = np.empty((NCHUNK, 128, D), dtype=np.float32)
    for c in range(NCHUNK):
        a, d = c // NJB, c % NJB
        w2v2[c] = W_perm[8 * a:8 * a + 8, 16 * d:16 * d + 16, :].reshape(128, D)
    w2 = w2v2.transpose(1, 0, 2).reshape(128, NCHUNK * D)
    w2 = np.ascontiguousarray(w2.astype(BF16NP))

    sel = np.zeros((NPAIR, 128, 128), dtype=BF16NP)
    for t in range(NPAIR):
        for half in range(2):
            a = 2 * t + half
            for i in range(8):
                for m in range(16):
                    sel[t, half * 64 + 8 * a + i, i * 16 + m] = 1.0
    sel = np.ascontiguousarray(sel.transpose(1, 0, 2).reshape(128, NPAIR * 128))
    return w2, sel


def kernel(atom_state, bond_state, connectivity, bond_transform,
           repeat=1, paths=PATHS_DEFAULT, deep=True):
    atom_state = np.asarray(atom_state, dtype=np.float32)
    bond_state = np.asarray(bond_state, dtype=np.float32)
    connectivity = np.asarray(connectivity)
    bond_transform = np.asarray(bond_transform, dtype=np.float32)

    nc = _get_nc(repeat, paths, deep)
    w2, sel = _shared_inputs(bond_transform)

    in_maps = []
    for m in range(NCORES):
        sl = slice(m * BPC, (m + 1) * BPC)
        atom8, bondT2, onehotT = _prep_core_inputs(
            atom_state[sl], bond_state[sl], connectivity[sl])
        in_maps.append({
            "atom8": atom8,
            "bondT2": bondT2,
            "onehotT": onehotT,
            "w2": w2,
            "sel": sel,
        })

    res = run_bass_kernel_spmd(nc, in_maps, list(range(NCORES)))

    out = np.empty((B, E, D), dtype=np.float32)
    for m in range(NCORES):
        msgT = np.asarray(res.results[m]["msgT"], dtype=np.float32)
        out[m * BPC:(m + 1) * BPC] = np.swapaxes(msgT, 1, 2)
    return out


if __name__ == "__main__":
    rng = np.random.default_rng(0)
    atom = rng.standard_normal((B, A, D)).astype(np.float32)
    bond = rng.standard_normal((B, E, K)).astype(np.float32)
    conn = rng.integers(0, A, size=(B, E, 2)).astype(np.int64)
    bt = rng.standard_normal((K, D * D)).astype(np.float32) * 0.01
    out = kernel(atom, bond, conn, bt)
    print("out", out.shape, out.dtype, float(np.abs(out).max()))
